# revision 100
# baseline (speedup 1.0000x reference)
"""Trainium2 Bass kernel for Ernie4.5-VL attention (mRoPE + GQA causal attention).

Sharding: tensor-parallel over heads across 8 cores. Each core computes
2 q heads + its kv head (replicated per core pair): qkv projection
(feature-major, bf16 inputs), interleaved mRoPE (host-precomputed cos/sin
tables + an even/odd column permutation of the q/k weight slices so the
rotation becomes two contiguous partition halves), causal attention with
unnormalized softmax (denominator via an all-ones matmul, normalization
after AV), and the o_proj partial product (bf16 out). Host sums the 8
partial outputs.

Matmul inputs are bf16 (same PE rate as f32r at free-dim 512, half the
DMA/SBUF traffic); PSUM accumulation is fp32 throughout. The attention
inner loop is software-pipelined: scores run PIPE tiles ahead of the AV
matmuls so the Exp (ACT) latency stays off the PE critical path.
"""
import numpy as np
import ml_dtypes
from contextlib import ExitStack

import concourse.bacc as bacc
import concourse.tile as tile
from concourse import mybir
from concourse.bass_utils import run_bass_kernel_spmd

HIDDEN = 2048
T = 2048
N_HEADS = 16
N_KV = 4
HD = 128
THETA = 500000.0
NCORES = 8
SCALE = HD ** -0.5

F32 = mybir.dt.float32
F32R = mybir.dt.float32r
BF16 = mybir.dt.bfloat16

# within-head column permutation: evens then odds (so interleaved rope pairs
# become two contiguous partition halves in feature-major layout)
PERM = np.concatenate([np.arange(0, HD, 2), np.arange(1, HD, 2)])
# pair index p (0..63): p<44: even->pos row 1 (h), odd->row 2 (w); p>=44: row 0 (t)
ROW_MAP = np.array([(1 if p % 2 == 0 else 2) if p < 44 else 0 for p in range(64)])
INVF = THETA ** (-(np.arange(64, dtype=np.float64) / 64))

NT = T // 128      # 16 token tiles
NG = T // 512      # 4 token chunks
NH_T = HIDDEN // 128  # 16 hidden tiles
PIPE = 3           # scores-ahead-of-AV pipeline depth


def _build():
    nc = bacc.Bacc("TRN2", target_bir_lowering=False, debug=False)
    d_xT = nc.dram_tensor("xT", [HIDDEN, T], BF16, kind="ExternalInput").ap()
    d_w = nc.dram_tensor("w_slice", [HIDDEN, 512], BF16, kind="ExternalInput").ap()
    d_wo = nc.dram_tensor("wo_slice", [256, HIDDEN], BF16, kind="ExternalInput").ap()
    d_cdup = nc.dram_tensor("cdup", [128, T], BF16, kind="ExternalInput").ap()
    d_sdup = nc.dram_tensor("sdup", [128, T], BF16, kind="ExternalInput").ap()
    d_mL = nc.dram_tensor("mask_l", [128, 128], BF16, kind="ExternalInput").ap()
    d_mR = nc.dram_tensor("mask_r", [128, 4, 512], BF16, kind="ExternalInput").ap()
    d_ones = nc.dram_tensor("ones", [128, 128], BF16, kind="ExternalInput").ap()
    d_yT = nc.dram_tensor("yT", [HIDDEN, T], BF16, kind="ExternalOutput").ap()

    with tile.TileContext(nc) as tc, ExitStack() as ctx:
        const = ctx.enter_context(tc.tile_pool(name="const", bufs=1))
        big = ctx.enter_context(tc.tile_pool(name="big", bufs=1))

        # resident tiles
        w_sb = const.tile([128, NH_T, 512], BF16)       # qkv weight slice
        wo_sb = const.tile([128, 2, HIDDEN], BF16)      # o_proj rows
        mL_sb = const.tile([128, 128], BF16)            # causal mask, left factor
        mR_sb = const.tile([128, 4, 512], BF16)         # causal mask, right factor
        ones_sb = const.tile([128, 128], BF16)
        cdup = const.tile([128, T], BF16)               # cos table (dup halves)
        sdup = const.tile([128, T], BF16)               # sin table ([-s; s])
        qkv_sb = big.tile([128, 3, T], BF16)            # q0|q1|k feature-major
        v_sb = big.tile([128, T], BF16)                 # v feature-major
        V_sb = big.tile([128, NT, 128], BF16)           # V token-major
        O_sb = big.tile([128, 2, T], BF16)              # attention out, feature-major

        # PSUM budget (8 banks): qkv accum 2 (two-pass) + shared
        # (scores/V-transpose/ones/o_proj) 4 + AV accum 2.
        xtp = ctx.enter_context(tc.tile_pool(name="xt", bufs=2))
        qkvp = ctx.enter_context(tc.tile_pool(name="qkvp", bufs=2, space="PSUM"))
        spp = ctx.enter_context(tc.tile_pool(name="spp", bufs=4, space="PSUM"))
        avp = ctx.enter_context(tc.tile_pool(name="avp", bufs=2, space="PSUM"))
        rp = ctx.enter_context(tc.tile_pool(name="rope", bufs=4))
        ep = ctx.enter_context(tc.tile_pool(name="ep", bufs=10))
        rv = ctx.enter_context(tc.tile_pool(name="rv", bufs=2))
        racc = ctx.enter_context(tc.tile_pool(name="racc", bufs=4))
        yo = ctx.enter_context(tc.tile_pool(name="yo", bufs=2))
        dum = ctx.enter_context(tc.tile_pool(name="dum", bufs=1))

        # Preload the ACT Exp table off the critical path (first real exp
        # otherwise pays the ~1.3us table load mid-attention).
        dummy = dum.tile([128, 8], F32)
        nc.vector.memset(dummy[:], 0.0)
        nc.scalar.activation(dummy[:], dummy[:],
                             mybir.ActivationFunctionType.Exp, scale=1.0)

        # ---- input loads. w + xT chunk 0 interleaved in quarters (the first
        # matmul only needs the first quarter of each), then tables ordered
        # by first use.
        def load_xt(g):
            # four quarter-DMAs, not one: a monolithic 5.8us transfer would
            # head-of-line block urgent small transfers (rope swaps) on the
            # serial DMA-engine resource
            xt_b = xtp.tile([128, NH_T, 512], BF16, tag="xt", name=f"xt_{g}")
            for q in range(4):
                nc.sync.dma_start(
                    out=xt_b[:, 4 * q:4 * (q + 1), :],
                    in_=d_xT[512 * q:512 * (q + 1),
                             512 * g:512 * (g + 1)].rearrange(
                        "(a p) c -> p a c", p=128))
            return xt_b

        # all preloads on the SP queue, ordered by first-use time (the DMA
        # engines drain one queue-arrival at a time, so order = need order)
        xt0 = xtp.tile([128, NH_T, 512], BF16, tag="xt", name="xt_0")
        pieces = [(0, 2), (2, 4), (4, 8), (8, 12), (12, 16)]
        for lo, hi in pieces:
            hs = np.s_[128 * lo:128 * hi]
            nc.sync.dma_start(
                out=w_sb[:, lo:hi, :],
                in_=d_w[hs, :].rearrange("(a p) c -> p a c", p=128))
            nc.sync.dma_start(
                out=xt0[:, lo:hi, :],
                in_=d_xT[hs, 0:512].rearrange("(a p) c -> p a c", p=128))
        xt_tiles = {0: xt0}
        nc.sync.dma_start(out=cdup, in_=d_cdup)
        nc.sync.dma_start(out=sdup, in_=d_sdup)
        nc.sync.dma_start(out=mL_sb, in_=d_mL)
        nc.sync.dma_start(out=mR_sb, in_=d_mR)
        nc.sync.dma_start(out=ones_sb, in_=d_ones)
        nc.sync.dma_start(out=wo_sb, in_=d_wo.rearrange("(a p) c -> p a c", p=128))

        def score_order(g):
            # (j, column slice start) in emission order: diag m=0 first
            # (full width), the full tiles, then the narrowing diagonals
            return [(4 * g, 0)] + [(j, 0) for j in range(4 * g)] + \
                   [(4 * g + m, 128 * m) for m in range(1, 4)]

        def emit_score(g, h, t, Es):
            """Scores+mask matmuls and exp for tile t of (g, h); E lands in
            Es[t]. Shared between attn() and the last chunk's early scores
            prefix (which runs between the qkv passes so ACT starts the
            tail's exp stream early)."""
            tsl = np.s_[512 * g:512 * (g + 1)]
            j, c0 = score_order(g)[t]
            m = j - 4 * g
            csl = np.s_[c0:512]
            ps = spp.tile([128, 512], F32, tag="sp", name=f"s{g}_{h}_{j}")
            nc.tensor.matmul(ps[:, csl],
                             qkv_sb[:, 2, 128 * j:128 * (j + 1)],
                             qkv_sb[:, h, tsl][:, csl],
                             start=True, stop=(m < 0),
                             skip_group_check=True)
            if m >= 0:
                # additive causal mask (-1e9 on invalid) via rank-factored
                # matmul; only the 128-wide triangular boundary block needs
                # it -- everything right of it is fully valid
                bsl = np.s_[128 * m:128 * (m + 1)]
                nc.tensor.matmul(ps[:, bsl], mL_sb[:], mR_sb[:, m, bsl],
                                 start=False, stop=True,
                                 skip_group_check=True)
            E = ep.tile([128, 512], BF16, tag="e", name=f"e{g}_{h}_{j}")
            nc.scalar.activation(E[:, csl], ps[:, csl],
                                 mybir.ActivationFunctionType.Exp,
                                 scale=SCALE)
            Es[t] = E

        def attn(g, filler=None, pre=None, on_head=None):
            """Attention for chunk g: scores+mask -> exp -> AV, pipelined.
            `filler` is an optional iterator of thunks whose instructions are
            interleaved after each AV to fill exp-latency bubbles. `pre` maps
            head -> {t: E} for score tiles already emitted upstream.

            Diagonal k-tile m only touches q columns [128m:512] (everything
            to its left is fully masked), so scores/mask/exp/AV/row-sums are
            column-sliced there. Tile order: diag m=0 first (full width,
            start=True clears the AV psum), then the full tiles, then the
            narrowing diagonals; accumulation-group flags are per-element
            nonconforming, hence skip_group_check."""
            tsl = np.s_[512 * g:512 * (g + 1)]
            order = score_order(g)
            n = len(order)
            ras = {}
            for h in range(2):
                po = avp.tile([128, 512], F32, tag="av", name=f"po{g}_{h}")
                ra = racc.tile([128, 512], BF16, tag="ra", name=f"ra{g}_{h}")
                rb = racc.tile([128, 512], BF16, tag="rb", name=f"rb{g}_{h}")
                rc = racc.tile([128, 512], BF16, tag="rc", name=f"rc{g}_{h}")
                Es = dict(pre.get(h, {})) if pre else {}

                def emit_s(t):
                    if t not in Es:
                        emit_score(g, h, t, Es)

                # row-sum chains: two on DVE (ra, rc) + one on Pool (rb);
                # the last tiles land on DVE so the slow Pool chain (1.1us
                # per add) never gates the denominator broadcast. Chain
                # heads are the widest tiles (t=0 is always full width).
                pool_ts = set(t for t in range(1, max(n - 2, 1)) if t % 3 == 1)
                cov = {}

                def emit_sum(t, E):
                    c0 = order[t][1]
                    csl = np.s_[c0:512]
                    if t in pool_ts:
                        key = "b"
                        acc = rb
                    elif t % 2 == 0:
                        key = "a"
                        acc = ra
                    else:
                        key = "c"
                        acc = rc
                    if key not in cov:
                        nc.gpsimd.tensor_copy(acc[:, csl], E[:, csl]) \
                            if key == "b" else \
                            nc.vector.tensor_copy(acc[:, csl], E[:, csl])
                        cov[key] = c0
                    else:
                        assert c0 >= cov[key]
                        if key == "b":
                            nc.gpsimd.tensor_add(acc[:, csl], acc[:, csl],
                                                 E[:, csl])
                        else:
                            nc.vector.tensor_add(acc[:, csl], acc[:, csl],
                                                 E[:, csl])

                def emit_av(t):
                    j, c0 = order[t]
                    csl = np.s_[c0:512]
                    E = Es.pop(t)
                    nc.tensor.matmul(po[:, csl], V_sb[:, j, :], E[:, csl],
                                     start=(t == 0), stop=(t == n - 1),
                                     skip_group_check=True)
                    emit_sum(t, E)

                for t in range(min(PIPE, n)):
                    emit_s(t)
                for t in range(n):
                    if t + PIPE < n:
                        emit_s(t + PIPE)
                    emit_av(t)
                    if filler is not None:
                        th = next(filler, None)
                        if th is not None:
                            th()
                if g < NG - 1:
                    # fold the side chains into ra on DVE; hidden under the
                    # next chunk's qkv matmuls
                    if "b" in cov:
                        bs = np.s_[cov["b"]:512]
                        nc.vector.tensor_add(ra[:, bs], ra[:, bs], rb[:, bs])
                    if "c" in cov:
                        cs = np.s_[cov["c"]:512]
                        nc.vector.tensor_add(ra[:, cs], ra[:, cs], rc[:, cs])
                    ras[h] = (po, ra, None, None, None)
                else:
                    # last chunk: nothing overlaps the merge chain, so defer
                    # the fold to accumulated ones-matmuls in finish_attn
                    ras[h] = (po, ra, rb, rc, dict(cov))
                if on_head is not None:
                    on_head(h, ras[h])
            return ras

        def finish_head(g, h, entry):
            """Denominator broadcast + normalization for one head."""
            tsl = np.s_[512 * g:512 * (g + 1)]
            if True:
                po, ra, rb, rc, cov = entry
                pr = spp.tile([128, 512], F32, tag="sp", name=f"pr{g}_{h}")
                if rb is None and rc is None and cov is None:
                    nc.tensor.matmul(pr[:], ones_sb[:], ra[:],
                                     start=True, stop=True)
                else:
                    # one accumulated ones-matmul per row-sum chain: each
                    # waits only its own chain, no serial DVE merges
                    parts = [(ra, 0)]
                    if "b" in cov:
                        parts.append((rb, cov["b"]))
                    if "c" in cov:
                        parts.append((rc, cov["c"]))
                    for pi, (acc, c0) in enumerate(parts):
                        csl = np.s_[c0:512]
                        nc.tensor.matmul(pr[:, csl], ones_sb[:], acc[:, csl],
                                         start=(pi == 0),
                                         stop=(pi == len(parts) - 1),
                                         skip_group_check=True)
                rinv = rv.tile([128, 512], F32, tag="rv", name=f"rinv{g}_{h}")
                nc.vector.reciprocal(rinv[:], pr[:])
                nc.vector.tensor_mul(O_sb[:, h, tsl], po[:], rinv[:])

        def finish_attn(g, ras):
            for h in range(2):
                finish_head(g, h, ras[h])

        def oproj_ops(g, psum_pool=None, psum_tag="sp", copies_on_act=True):
            """o_proj partial chunk, as a list of single-instruction thunks
            (so the tail can interleave them into attention's exp bubbles).
            Head 1 (whose attention ran first, so its normalization chain
            drained first) leads; head 0 lags by 2 tiles so its chain stays
            off the PE critical path."""
            tsl = np.s_[512 * g:512 * (g + 1)]
            yt = yo.tile([128, NH_T, 512], BF16, tag="yo", name=f"yt{g}")
            pys = {}
            LEAD, LAG = 0, 1
            ops = []
            last = (g == NG - 1)

            def pick_pool(i):
                if psum_pool is not None:
                    return psum_pool, psum_tag
                if last and i % 3 == 2:
                    # the final o_proj can span 6 psum banks (scores and the
                    # filler are drained): 4 from spp + 2 from qkvp, enabling
                    # a deep LEAD prefix that hides head-1's normalize chain
                    return qkvp, "qkvps"
                return spp, "sp"

            def emit_mm(h, i):
                def th():
                    if h == LEAD:
                        pl, tg = pick_pool(i)
                        py = pl.tile([128, 512], F32, tag=tg,
                                     name=f"y{g}_{i}")
                        pys[i] = py
                    else:
                        py = pys[i]
                    nc.tensor.matmul(py[:], wo_sb[:, h, 128 * i:128 * (i + 1)],
                                     O_sb[:, h, tsl], start=(h == LEAD),
                                     stop=(h == LAG))
                ops.append(th)

            def emit_copy(i):
                def th():
                    py = pys.pop(i)
                    if i % 2 == 1 and copies_on_act:
                        nc.scalar.copy(yt[:, i, :], py[:])
                    else:
                        nc.vector.tensor_copy(yt[:, i, :], py[:])
                ops.append(th)

            def emit_store(lo, hi):
                def th():
                    nc.sync.dma_start(
                        out=d_yT[128 * lo:128 * hi, tsl].rearrange(
                            "(a p) c -> p a c", p=128),
                        in_=yt[:, lo:hi, :])
                ops.append(th)

            # last chunk: deeper LEAD prefix (6 psums) so the LAG stream's
            # wait on head-1's normalize chain (ones -> recip -> mul) hides
            lag = 6 if last else 2
            for i in range(lag):
                emit_mm(LEAD, i)
            for i in range(NH_T):
                if i + lag < NH_T:
                    emit_mm(LEAD, i + lag)
                emit_mm(LAG, i)
                emit_copy(i)
                if g == NG - 1 and i % 2 == 1:
                    emit_store(i - 1, i + 1)
                elif g < NG - 1 and i % 4 == 3:
                    emit_store(i - 3, i + 1)
            return ops

        def oproj(g):
            for th in oproj_ops(g):
                th()

        # main loop, software-pipelined one chunk deep: while chunk g's
        # projection + rope run (DMA/DVE-heavy), the PE executes chunk g-1's
        # attention + o_proj.
        prev_ras = None
        NPRE = 6
        pre_es = {}
        for g in range(NG):
            tsl = np.s_[512 * g:512 * (g + 1)]
            xt_b = xt_tiles.pop(g)

            # ---- previous chunk's attention FIRST: its end-of-chunk
            # normalization chains (DVE row-sum merges -> ones -> recip ->
            # mul) then drain under this chunk's qkv matmuls instead of
            # stalling the PE before o_proj
            if g + 1 < NG:
                xt_tiles[g + 1] = load_xt(g + 1)
            if prev_ras is not None:
                ras = attn(g - 1)
                finish_attn(g - 1, ras)

            # ---- qkv projection chunk, feature-major, in passes. Each
            # projected tensor gets its psum->SBUF copy, half-swap DMA, and
            # rope emitted as soon as its pass finishes (k first: it gates
            # the next chunk's scores). Chunk 0 has no attention to overlap,
            # so it front-loads (k, v, q0) in a 3-psum pass (borrowing an
            # attention psum slot) -- q0's rope then unblocks attention(0)
            # head-0 while q1 is still projecting.
            def copy_kv(psk, psv):
                # k on DVE, v on ACT: the two copies run in parallel, halving
                # the latency until the next pass's psum banks free up
                nc.vector.tensor_copy(qkv_sb[:, 2, tsl], psk[:])
                nc.scalar.copy(v_sb[:, tsl], psv[:])
                xs_k = rp.tile([128, 512], BF16, tag="xsk", name=f"xsk{g}")
                nc.sync.dma_start(out=xs_k[0:64, :], in_=qkv_sb[64:128, 2, tsl])
                nc.sync.dma_start(out=xs_k[64:128, :], in_=qkv_sb[0:64, 2, tsl])
                return xs_k

            def copy_q(ps0, ps1):
                nc.vector.tensor_copy(qkv_sb[:, 0, tsl], ps0[:])
                nc.scalar.copy(qkv_sb[:, 1, tsl], ps1[:])
                xs_q = rp.tile([128, 2, 512], BF16, tag="xsq", name=f"xsq{g}")
                nc.sync.dma_start(out=xs_q[0:64, :, :],
                                  in_=qkv_sb[64:128, 0:2, tsl])
                nc.sync.dma_start(out=xs_q[64:128, :, :],
                                  in_=qkv_sb[0:64, 0:2, tsl])
                return xs_q

            def qkv_pass(cols, tag_i):
                psums = [qkvp.tile([128, 512], F32, tag="qkvps",
                                   name=f"qkvps_{g}_{tag_i}_{i}")
                         for i in range(len(cols))]
                for h in range(NH_T):
                    for pi, i in enumerate(cols):
                        nc.tensor.matmul(
                            psums[pi][:], w_sb[:, h, 128 * i:128 * (i + 1)],
                            xt_b[:, h, :],
                            start=(h == 0), stop=(h == NH_T - 1))
                return psums

            def swap1(c, ps, on_act=False):
                # psum->SBUF copy + partition half-swap for one rope tensor
                if on_act:
                    nc.scalar.copy(qkv_sb[:, c, tsl], ps[:])
                else:
                    nc.vector.tensor_copy(qkv_sb[:, c, tsl], ps[:])
                xs = rp.tile([128, 512], BF16, tag=f"xs1_{c}",
                             name=f"xs1_{g}_{c}")
                nc.sync.dma_start(out=xs[0:64, :], in_=qkv_sb[64:128, c, tsl])
                nc.sync.dma_start(out=xs[64:128, :], in_=qkv_sb[0:64, c, tsl])
                return xs

            def rope1(c, xs):
                x = qkv_sb[:, c, tsl]
                t1 = rp.tile([128, 512], BF16, tag="t1", name=f"t1_{g}_{c}")
                t2 = rp.tile([128, 512], BF16, tag="t2", name=f"t2_{g}_{c}")
                nc.vector.tensor_mul(t1[:], x, cdup[:, tsl])
                nc.vector.tensor_mul(t2[:], xs[:], sdup[:, tsl])
                nc.vector.tensor_add(x, t1[:], t2[:])

            if g == NG - 1:
                # last chunk: project (k, q0) first, rope them, and pre-emit
                # the first NPRE score+exp tiles of head 0 so ACT starts the
                # tail's exp stream (the end-time critical chain) while the
                # (v, q1) pass still runs on the PE. No AVs here: they would
                # deadlock the PE FIFO on the V transpose behind pass 1.
                ps0 = qkv_pass((2, 0), 0)
                xs_k1 = swap1(2, ps0[0])
                xs_q0 = swap1(0, ps0[1], on_act=True)
                rope1(2, xs_k1)
                rope1(0, xs_q0)
                for t in range(NPRE):
                    emit_score(g, 0, t, pre_es)
                ps1 = qkv_pass((3, 1), 1)
                nc.vector.tensor_copy(v_sb[:, tsl], ps1[0][:])
                nc.sync.dma_start_transpose(
                    out=V_sb[:, 4 * g:4 * (g + 1), :], in_=v_sb[:, tsl])
                xs_q1 = swap1(1, ps1[1], on_act=True)
                rope1(1, xs_q1)
            elif g == 0:
                # q0/q1 psums borrow attention slots; k/v take the qkvp
                # banks, whose copies run first -- so chunk 1's projection
                # (which reuses qkvp) unblocks as early as possible
                psums = [spp.tile([128, 512], F32, tag="sp",
                                  name=f"qkvps_0_b_{i}") for i in range(2)]
                psums += [qkvp.tile([128, 512], F32, tag="qkvps",
                                    name=f"qkvps_0_a_{i}") for i in range(2)]
                for h in range(NH_T):
                    for i in range(4):
                        nc.tensor.matmul(
                            psums[i][:], w_sb[:, h, 128 * i:128 * (i + 1)],
                            xt_b[:, h, :],
                            start=(h == 0), stop=(h == NH_T - 1))
                xs_k = copy_kv(psums[2], psums[3])
                xs_q = copy_q(psums[0], psums[1])
            else:
                for p in range(2):
                    psums = [qkvp.tile([128, 512], F32, tag="qkvps",
                                       name=f"qkvps_{g}_{p}_{i}")
                             for i in range(2)]
                    cols = (2, 3) if p == 0 else (0, 1)
                    for h in range(NH_T):
                        for pi, i in enumerate(cols):
                            nc.tensor.matmul(
                                psums[pi][:], w_sb[:, h, 128 * i:128 * (i + 1)],
                                xt_b[:, h, :],
                                start=(h == 0), stop=(h == NH_T - 1))
                    if p == 0:
                        xs_k = copy_kv(psums[0], psums[1])
                    else:
                        xs_q = copy_q(psums[0], psums[1])

            if g < NG - 1:
                # ---- V transpose via the DMA XBAR (bf16): no PE work, no
                # psum contention with o_proj, no DVE copies
                nc.sync.dma_start_transpose(
                    out=V_sb[:, 4 * g:4 * (g + 1), :], in_=v_sb[:, tsl])

                # ---- rope (k first: it gates the next chunk's scores)
                for t3 in (2, 0, 1):
                    x = qkv_sb[:, t3, tsl]
                    xs = xs_k[:, :] if t3 == 2 else xs_q[:, t3, :]
                    t1 = rp.tile([128, 512], BF16, tag="t1",
                                 name=f"t1_{g}_{t3}")
                    t2 = rp.tile([128, 512], BF16, tag="t2",
                                 name=f"t2_{g}_{t3}")
                    nc.vector.tensor_mul(t1[:], x, cdup[:, tsl])
                    nc.vector.tensor_mul(t2[:], xs, sdup[:, tsl])
                    nc.vector.tensor_add(x, t1[:], t2[:])

            if prev_ras is not None and g < NG - 1:
                for th in oproj_ops(g - 1, copies_on_act=False):
                    th()
            prev_ras = g

        # tail: chunk NG-2's o_proj is interleaved into chunk NG-1's
        # attention (which is otherwise exp-throughput-bound with nothing
        # else to run); its psums use the now-free qkv banks
        fill = iter(oproj_ops(NG - 2, psum_pool=qkvp, psum_tag="qkvps",
                              copies_on_act=False))
        def on_head(h, entry):
            # head 0's normalize chain emitted between the heads: its ones/
            # recip/mul drain under head 1's attention, so o_proj's LEAD
            # stream starts the moment the last AV lands
            if h == 0:
                finish_head(NG - 1, 0, entry)
        ras = attn(NG - 1, filler=fill, pre={0: pre_es}, on_head=on_head)
        for th in fill:
            th()
        finish_head(NG - 1, 1, ras[1])
        oproj(NG - 1)

    nc.compile()
    return nc


_NC_CACHE = None


def _get_nc():
    global _NC_CACHE
    if _NC_CACHE is None:
        _NC_CACHE = _build()
    return _NC_CACHE


def _host_prep(positions, hidden_states, w_qkv, w_o):
    positions = np.asarray(positions, dtype=np.int32)
    hidden_states = np.asarray(hidden_states, dtype=np.float32)
    w_qkv = np.asarray(w_qkv, dtype=np.float32)
    w_o = np.asarray(w_o, dtype=np.float32)
    bf = ml_dtypes.bfloat16

    xT = np.ascontiguousarray(hidden_states.T).astype(bf)

    # rope tables: partition p holds pair p%64; lower half is the x1 (even)
    # feature, upper half the x2 (odd) feature of each rotary pair
    ang = positions[ROW_MAP, :].astype(np.float64) * INVF[:, None]
    cos = np.cos(ang).astype(np.float32)
    sin = np.sin(ang).astype(np.float32)
    cdup = np.ascontiguousarray(np.concatenate([cos, cos], axis=0)).astype(bf)
    sdup = np.ascontiguousarray(np.concatenate([-sin, sin], axis=0)).astype(bf)

    # additive causal mask factors: invalid(dk, dq) = [dq - 128m + 1 <= dk]
    #   = sum_p L[p, dk] * Rm[p, dq],  L[p, dk] = [p <= dk],
    #   Rm[p, dq] = [p == max(dq - 128m + 1, 0)]  (scaled by -1e9)
    mask_l = (np.arange(128)[:, None] <= np.arange(128)[None, :]).astype(np.float32)
    mask_r = np.zeros((128, 4, 512), dtype=np.float32)
    for m in range(4):
        c = np.maximum(np.arange(512) - 128 * m + 1, 0)
        valid_rows = c <= 127
        mask_r[c[valid_rows], m, np.arange(512)[valid_rows]] = -1e9
    ones = np.ones((128, 128), dtype=np.float32)

    q_size = N_HEADS * HD
    kv_size = N_KV * HD
    in_maps = []
    for c in range(NCORES):
        cols = [w_qkv[:, 2 * c * HD + PERM], w_qkv[:, (2 * c + 1) * HD + PERM]]
        kc = c // 2
        cols.append(w_qkv[:, q_size + kc * HD + PERM])
        cols.append(w_qkv[:, q_size + kv_size + kc * HD:q_size + kv_size + (kc + 1) * HD])
        w_slice = np.ascontiguousarray(np.concatenate(cols, axis=1)).astype(bf)
        wo_slice = np.ascontiguousarray(w_o[2 * c * HD:(2 * c + 2) * HD]).astype(bf)
        in_maps.append({
            "xT": xT, "w_slice": w_slice, "wo_slice": wo_slice,
            "cdup": cdup, "sdup": sdup,
            "mask_l": mask_l.astype(bf), "mask_r": mask_r.astype(bf),
            "ones": ones.astype(bf),
        })
    return in_maps


def kernel(positions, hidden_states, w_qkv, w_o):
    nc = _get_nc()
    in_maps = _host_prep(positions, hidden_states, w_qkv, w_o)
    # one retry: transient NRT/device errors (e.g. NRT_EXEC_UNIT_UNRECOVERABLE
    # from a wedged core) were observed to succeed on re-dispatch
    try:
        res = run_bass_kernel_spmd(nc, in_maps, core_ids=list(range(NCORES)))
    except Exception:
        import time
        time.sleep(2.0)
        res = run_bass_kernel_spmd(nc, in_maps, core_ids=list(range(NCORES)))
    yT = np.zeros((HIDDEN, T), dtype=np.float64)
    for c in range(NCORES):
        yT += res.results[c]["yT"].astype(np.float64)
    return np.ascontiguousarray(yT.T).astype(np.float32)


# revision 103
# speedup vs baseline: 1.0039x; 1.0039x over previous
"""Trainium2 Bass kernel for Ernie4.5-VL attention (mRoPE + GQA causal attention).

Sharding: tensor-parallel over heads across 8 cores. Each core computes
2 q heads + its kv head (replicated per core pair): qkv projection
(feature-major, bf16 inputs), interleaved mRoPE (host-precomputed cos/sin
tables + an even/odd column permutation of the q/k weight slices so the
rotation becomes two contiguous partition halves), causal attention with
unnormalized softmax (denominator via an all-ones matmul, normalization
after AV), and the o_proj partial product (bf16 out). Host sums the 8
partial outputs.

Matmul inputs are bf16 (same PE rate as f32r at free-dim 512, half the
DMA/SBUF traffic); PSUM accumulation is fp32 throughout. The attention
inner loop is software-pipelined: scores run PIPE tiles ahead of the AV
matmuls so the Exp (ACT) latency stays off the PE critical path.
"""
import numpy as np
import ml_dtypes
from contextlib import ExitStack

import concourse.bacc as bacc
import concourse.tile as tile
from concourse import mybir
from concourse.bass_utils import run_bass_kernel_spmd

HIDDEN = 2048
T = 2048
N_HEADS = 16
N_KV = 4
HD = 128
THETA = 500000.0
NCORES = 8
SCALE = HD ** -0.5

F32 = mybir.dt.float32
F32R = mybir.dt.float32r
BF16 = mybir.dt.bfloat16

# within-head column permutation: evens then odds (so interleaved rope pairs
# become two contiguous partition halves in feature-major layout)
PERM = np.concatenate([np.arange(0, HD, 2), np.arange(1, HD, 2)])
# pair index p (0..63): p<44: even->pos row 1 (h), odd->row 2 (w); p>=44: row 0 (t)
ROW_MAP = np.array([(1 if p % 2 == 0 else 2) if p < 44 else 0 for p in range(64)])
INVF = THETA ** (-(np.arange(64, dtype=np.float64) / 64))

NT = T // 128      # 16 token tiles
NG = T // 512      # 4 token chunks
NH_T = HIDDEN // 128  # 16 hidden tiles
PIPE = 3           # scores-ahead-of-AV pipeline depth


def _build():
    nc = bacc.Bacc("TRN2", target_bir_lowering=False, debug=False)
    d_xT = nc.dram_tensor("xT", [HIDDEN, T], BF16, kind="ExternalInput").ap()
    d_w = nc.dram_tensor("w_slice", [HIDDEN, 512], BF16, kind="ExternalInput").ap()
    d_wo = nc.dram_tensor("wo_slice", [256, HIDDEN], BF16, kind="ExternalInput").ap()
    d_cdup = nc.dram_tensor("cdup", [128, T], BF16, kind="ExternalInput").ap()
    d_sdup = nc.dram_tensor("sdup", [128, T], BF16, kind="ExternalInput").ap()
    d_mL = nc.dram_tensor("mask_l", [128, 128], BF16, kind="ExternalInput").ap()
    d_mR = nc.dram_tensor("mask_r", [128, 4, 512], BF16, kind="ExternalInput").ap()
    d_ones = nc.dram_tensor("ones", [128, 128], BF16, kind="ExternalInput").ap()
    d_yT = nc.dram_tensor("yT", [HIDDEN, T], BF16, kind="ExternalOutput").ap()

    with tile.TileContext(nc) as tc, ExitStack() as ctx:
        const = ctx.enter_context(tc.tile_pool(name="const", bufs=1))
        big = ctx.enter_context(tc.tile_pool(name="big", bufs=1))

        # resident tiles
        w_sb = const.tile([128, NH_T, 512], BF16)       # qkv weight slice
        wo_sb = const.tile([128, 2, HIDDEN], BF16)      # o_proj rows
        mL_sb = const.tile([128, 128], BF16)            # causal mask, left factor
        mR_sb = const.tile([128, 4, 512], BF16)         # causal mask, right factor
        ones_sb = const.tile([128, 128], BF16)
        cdup = const.tile([128, T], BF16)               # cos table (dup halves)
        sdup = const.tile([128, T], BF16)               # sin table ([-s; s])
        qkv_sb = big.tile([128, 3, T], BF16)            # q0|q1|k feature-major
        v_sb = big.tile([128, T], BF16)                 # v feature-major
        V_sb = big.tile([128, NT, 128], BF16)           # V token-major
        O_sb = big.tile([128, 2, T], BF16)              # attention out, feature-major

        # PSUM budget (8 banks): qkv accum 2 (two-pass) + shared
        # (scores/V-transpose/ones/o_proj) 4 + AV accum 2.
        xtp = ctx.enter_context(tc.tile_pool(name="xt", bufs=2))
        qkvp = ctx.enter_context(tc.tile_pool(name="qkvp", bufs=2, space="PSUM"))
        spp = ctx.enter_context(tc.tile_pool(name="spp", bufs=4, space="PSUM"))
        avp = ctx.enter_context(tc.tile_pool(name="avp", bufs=2, space="PSUM"))
        rp = ctx.enter_context(tc.tile_pool(name="rope", bufs=4))
        ep = ctx.enter_context(tc.tile_pool(name="ep", bufs=10))
        rv = ctx.enter_context(tc.tile_pool(name="rv", bufs=2))
        racc = ctx.enter_context(tc.tile_pool(name="racc", bufs=4))
        yo = ctx.enter_context(tc.tile_pool(name="yo", bufs=2))
        dum = ctx.enter_context(tc.tile_pool(name="dum", bufs=1))

        # Preload the ACT Exp table off the critical path (first real exp
        # otherwise pays the ~1.3us table load mid-attention).
        dummy = dum.tile([128, 8], F32)
        nc.vector.memset(dummy[:], 0.0)
        nc.scalar.activation(dummy[:], dummy[:],
                             mybir.ActivationFunctionType.Exp, scale=1.0)

        # ---- input loads. w + xT chunk 0 interleaved in quarters (the first
        # matmul only needs the first quarter of each), then tables ordered
        # by first use.
        def load_xt(g):
            # four quarter-DMAs, not one: a monolithic 5.8us transfer would
            # head-of-line block urgent small transfers (rope swaps) on the
            # serial DMA-engine resource
            xt_b = xtp.tile([128, NH_T, 512], BF16, tag="xt", name=f"xt_{g}")
            for q in range(4):
                nc.sync.dma_start(
                    out=xt_b[:, 4 * q:4 * (q + 1), :],
                    in_=d_xT[512 * q:512 * (q + 1),
                             512 * g:512 * (g + 1)].rearrange(
                        "(a p) c -> p a c", p=128))
            return xt_b

        # all preloads on the SP queue, ordered by first-use time (the DMA
        # engines drain one queue-arrival at a time, so order = need order)
        xt0 = xtp.tile([128, NH_T, 512], BF16, tag="xt", name="xt_0")
        pieces = [(0, 2), (2, 4), (4, 8), (8, 12), (12, 16)]
        for lo, hi in pieces:
            hs = np.s_[128 * lo:128 * hi]
            nc.sync.dma_start(
                out=w_sb[:, lo:hi, :],
                in_=d_w[hs, :].rearrange("(a p) c -> p a c", p=128))
            nc.sync.dma_start(
                out=xt0[:, lo:hi, :],
                in_=d_xT[hs, 0:512].rearrange("(a p) c -> p a c", p=128))
        xt_tiles = {0: xt0}
        nc.sync.dma_start(out=cdup, in_=d_cdup)
        nc.sync.dma_start(out=sdup, in_=d_sdup)
        nc.sync.dma_start(out=mL_sb, in_=d_mL)
        nc.sync.dma_start(out=mR_sb, in_=d_mR)
        nc.sync.dma_start(out=ones_sb, in_=d_ones)
        nc.sync.dma_start(out=wo_sb, in_=d_wo.rearrange("(a p) c -> p a c", p=128))

        def score_order(g):
            # (j, column slice start) in emission order: diag m=0 first
            # (full width), the full tiles, then the narrowing diagonals
            return [(4 * g, 0)] + [(j, 0) for j in range(4 * g)] + \
                   [(4 * g + m, 128 * m) for m in range(1, 4)]

        def emit_score(g, h, t, Es):
            """Scores+mask matmuls and exp for tile t of (g, h); E lands in
            Es[t]. Shared between attn() and the last chunk's early scores
            prefix (which runs between the qkv passes so ACT starts the
            tail's exp stream early)."""
            tsl = np.s_[512 * g:512 * (g + 1)]
            j, c0 = score_order(g)[t]
            m = j - 4 * g
            csl = np.s_[c0:512]
            ps = spp.tile([128, 512], F32, tag="sp", name=f"s{g}_{h}_{j}")
            nc.tensor.matmul(ps[:, csl],
                             qkv_sb[:, 2, 128 * j:128 * (j + 1)],
                             qkv_sb[:, h, tsl][:, csl],
                             start=True, stop=(m < 0),
                             skip_group_check=True)
            if m >= 0:
                # additive causal mask (-1e9 on invalid) via rank-factored
                # matmul; only the 128-wide triangular boundary block needs
                # it -- everything right of it is fully valid
                bsl = np.s_[128 * m:128 * (m + 1)]
                nc.tensor.matmul(ps[:, bsl], mL_sb[:], mR_sb[:, m, bsl],
                                 start=False, stop=True,
                                 skip_group_check=True)
            E = ep.tile([128, 512], BF16, tag="e", name=f"e{g}_{h}_{j}")
            nc.scalar.activation(E[:, csl], ps[:, csl],
                                 mybir.ActivationFunctionType.Exp,
                                 scale=SCALE)
            Es[t] = E

        def attn(g, filler=None, pre=None, on_head=None):
            """Attention for chunk g: scores+mask -> exp -> AV, pipelined.
            `filler` is an optional iterator of thunks whose instructions are
            interleaved after each AV to fill exp-latency bubbles. `pre` maps
            head -> {t: E} for score tiles already emitted upstream.

            Diagonal k-tile m only touches q columns [128m:512] (everything
            to its left is fully masked), so scores/mask/exp/AV/row-sums are
            column-sliced there. Tile order: diag m=0 first (full width,
            start=True clears the AV psum), then the full tiles, then the
            narrowing diagonals; accumulation-group flags are per-element
            nonconforming, hence skip_group_check."""
            tsl = np.s_[512 * g:512 * (g + 1)]
            order = score_order(g)
            n = len(order)
            ras = {}
            for h in range(2):
                po = avp.tile([128, 512], F32, tag="av", name=f"po{g}_{h}")
                ra = racc.tile([128, 512], BF16, tag="ra", name=f"ra{g}_{h}")
                rb = racc.tile([128, 512], BF16, tag="rb", name=f"rb{g}_{h}")
                rc = racc.tile([128, 512], BF16, tag="rc", name=f"rc{g}_{h}")
                Es = dict(pre.get(h, {})) if pre else {}

                def emit_s(t):
                    if t not in Es:
                        emit_score(g, h, t, Es)

                # row-sum chains: two on DVE (ra, rc) + one on Pool (rb);
                # the last tiles land on DVE so the slow Pool chain (1.1us
                # per add) never gates the denominator broadcast. Chain
                # heads are the widest tiles (t=0 is always full width).
                pool_ts = set(t for t in range(1, max(n - 2, 1)) if t % 3 == 1)
                cov = {}

                def emit_sum(t, E):
                    c0 = order[t][1]
                    csl = np.s_[c0:512]
                    if t in pool_ts:
                        key = "b"
                        acc = rb
                    elif t % 2 == 0:
                        key = "a"
                        acc = ra
                    else:
                        key = "c"
                        acc = rc
                    if key not in cov:
                        nc.gpsimd.tensor_copy(acc[:, csl], E[:, csl]) \
                            if key == "b" else \
                            nc.vector.tensor_copy(acc[:, csl], E[:, csl])
                        cov[key] = c0
                    else:
                        assert c0 >= cov[key]
                        if key == "b":
                            nc.gpsimd.tensor_add(acc[:, csl], acc[:, csl],
                                                 E[:, csl])
                        else:
                            nc.vector.tensor_add(acc[:, csl], acc[:, csl],
                                                 E[:, csl])

                def emit_av(t):
                    j, c0 = order[t]
                    csl = np.s_[c0:512]
                    E = Es.pop(t)
                    nc.tensor.matmul(po[:, csl], V_sb[:, j, :], E[:, csl],
                                     start=(t == 0), stop=(t == n - 1),
                                     skip_group_check=True)
                    emit_sum(t, E)

                for t in range(min(PIPE, n)):
                    emit_s(t)
                for t in range(n):
                    if t + PIPE < n:
                        emit_s(t + PIPE)
                    emit_av(t)
                    if filler is not None:
                        th = next(filler, None)
                        if th is not None:
                            th()
                if g < NG - 1:
                    # fold the side chains into ra on DVE; hidden under the
                    # next chunk's qkv matmuls
                    if "b" in cov:
                        bs = np.s_[cov["b"]:512]
                        nc.vector.tensor_add(ra[:, bs], ra[:, bs], rb[:, bs])
                    if "c" in cov:
                        cs = np.s_[cov["c"]:512]
                        nc.vector.tensor_add(ra[:, cs], ra[:, cs], rc[:, cs])
                    ras[h] = (po, ra, None, None, None)
                else:
                    # last chunk: nothing overlaps the merge chain, so defer
                    # the fold to accumulated ones-matmuls in finish_attn
                    ras[h] = (po, ra, rb, rc, dict(cov))
                if on_head is not None:
                    on_head(h, ras[h])
            return ras

        def finish_head(g, h, entry):
            """Denominator broadcast + normalization for one head."""
            tsl = np.s_[512 * g:512 * (g + 1)]
            if True:
                po, ra, rb, rc, cov = entry
                pr = spp.tile([128, 512], F32, tag="sp", name=f"pr{g}_{h}")
                if rb is None and rc is None and cov is None:
                    nc.tensor.matmul(pr[:], ones_sb[:], ra[:],
                                     start=True, stop=True)
                else:
                    # one accumulated ones-matmul per row-sum chain: each
                    # waits only its own chain, no serial DVE merges
                    parts = [(ra, 0)]
                    if "b" in cov:
                        parts.append((rb, cov["b"]))
                    if "c" in cov:
                        parts.append((rc, cov["c"]))
                    for pi, (acc, c0) in enumerate(parts):
                        csl = np.s_[c0:512]
                        nc.tensor.matmul(pr[:, csl], ones_sb[:], acc[:, csl],
                                         start=(pi == 0),
                                         stop=(pi == len(parts) - 1),
                                         skip_group_check=True)
                rinv = rv.tile([128, 512], F32, tag="rv", name=f"rinv{g}_{h}")
                nc.vector.reciprocal(rinv[:], pr[:])
                nc.vector.tensor_mul(O_sb[:, h, tsl], po[:], rinv[:])

        def finish_attn(g, ras):
            for h in range(2):
                finish_head(g, h, ras[h])

        def oproj_ops(g, psum_pool=None, psum_tag="sp", copies_on_act=True):
            """o_proj partial chunk, as a list of single-instruction thunks
            (so the tail can interleave them into attention's exp bubbles).
            Head 1 (whose attention ran first, so its normalization chain
            drained first) leads; head 0 lags by 2 tiles so its chain stays
            off the PE critical path."""
            tsl = np.s_[512 * g:512 * (g + 1)]
            yt = yo.tile([128, NH_T, 512], BF16, tag="yo", name=f"yt{g}")
            pys = {}
            LEAD, LAG = 0, 1
            ops = []
            last = (g == NG - 1)

            def pick_pool(i):
                if psum_pool is not None:
                    return psum_pool, psum_tag
                if last and i % 3 == 2:
                    # the final o_proj can span 6 psum banks (scores and the
                    # filler are drained): 4 from spp + 2 from qkvp, enabling
                    # a deep LEAD prefix that hides head-1's normalize chain
                    return qkvp, "qkvps"
                return spp, "sp"

            def emit_mm(h, i):
                def th():
                    if h == LEAD:
                        pl, tg = pick_pool(i)
                        py = pl.tile([128, 512], F32, tag=tg,
                                     name=f"y{g}_{i}")
                        pys[i] = py
                    else:
                        py = pys[i]
                    nc.tensor.matmul(py[:], wo_sb[:, h, 128 * i:128 * (i + 1)],
                                     O_sb[:, h, tsl], start=(h == LEAD),
                                     stop=(h == LAG))
                ops.append(th)

            def emit_copy(i):
                def th():
                    py = pys.pop(i)
                    if i % 2 == 1 and copies_on_act:
                        nc.scalar.copy(yt[:, i, :], py[:])
                    else:
                        nc.vector.tensor_copy(yt[:, i, :], py[:])
                ops.append(th)

            def emit_store(lo, hi):
                def th():
                    nc.sync.dma_start(
                        out=d_yT[128 * lo:128 * hi, tsl].rearrange(
                            "(a p) c -> p a c", p=128),
                        in_=yt[:, lo:hi, :])
                ops.append(th)

            # last chunk: deeper LEAD prefix (6 psums) so the LAG stream's
            # wait on head-1's normalize chain (ones -> recip -> mul) hides
            lag = 6 if last else 2
            for i in range(lag):
                emit_mm(LEAD, i)
            for i in range(NH_T):
                if i + lag < NH_T:
                    emit_mm(LEAD, i + lag)
                emit_mm(LAG, i)
                emit_copy(i)
                if g == NG - 1 and i % 2 == 1:
                    emit_store(i - 1, i + 1)
                elif g < NG - 1 and i % 4 == 3:
                    emit_store(i - 3, i + 1)
            return ops

        def oproj(g):
            for th in oproj_ops(g):
                th()

        # main loop, software-pipelined one chunk deep: while chunk g's
        # projection + rope run (DMA/DVE-heavy), the PE executes chunk g-1's
        # attention + o_proj.
        prev_ras = None
        NPRE = 6
        pre_es = {}
        for g in range(NG):
            tsl = np.s_[512 * g:512 * (g + 1)]
            xt_b = xt_tiles.pop(g)

            # ---- previous chunk's attention FIRST: its end-of-chunk
            # normalization chains (DVE row-sum merges -> ones -> recip ->
            # mul) then drain under this chunk's qkv matmuls instead of
            # stalling the PE before o_proj
            if g + 1 < NG:
                xt_tiles[g + 1] = load_xt(g + 1)
            if prev_ras is not None:
                ras = attn(g - 1)
                finish_attn(g - 1, ras)

            # ---- qkv projection chunk, feature-major, in passes. Each
            # projected tensor gets its psum->SBUF copy, half-swap DMA, and
            # rope emitted as soon as its pass finishes (k first: it gates
            # the next chunk's scores). Chunk 0 has no attention to overlap,
            # so it front-loads (k, v, q0) in a 3-psum pass (borrowing an
            # attention psum slot) -- q0's rope then unblocks attention(0)
            # head-0 while q1 is still projecting.
            def copy_kv(psk, psv):
                # k on DVE, v on ACT: the two copies run in parallel, halving
                # the latency until the next pass's psum banks free up
                nc.vector.tensor_copy(qkv_sb[:, 2, tsl], psk[:])
                nc.scalar.copy(v_sb[:, tsl], psv[:])
                xs_k = rp.tile([128, 512], BF16, tag="xsk", name=f"xsk{g}")
                nc.sync.dma_start(out=xs_k[0:64, :], in_=qkv_sb[64:128, 2, tsl])
                nc.sync.dma_start(out=xs_k[64:128, :], in_=qkv_sb[0:64, 2, tsl])
                return xs_k

            def copy_q(ps0, ps1):
                nc.vector.tensor_copy(qkv_sb[:, 0, tsl], ps0[:])
                nc.scalar.copy(qkv_sb[:, 1, tsl], ps1[:])
                xs_q = rp.tile([128, 2, 512], BF16, tag="xsq", name=f"xsq{g}")
                nc.sync.dma_start(out=xs_q[0:64, :, :],
                                  in_=qkv_sb[64:128, 0:2, tsl])
                nc.sync.dma_start(out=xs_q[64:128, :, :],
                                  in_=qkv_sb[0:64, 0:2, tsl])
                return xs_q

            def qkv_pass(cols, tag_i):
                psums = [qkvp.tile([128, 512], F32, tag="qkvps",
                                   name=f"qkvps_{g}_{tag_i}_{i}")
                         for i in range(len(cols))]
                for h in range(NH_T):
                    for pi, i in enumerate(cols):
                        nc.tensor.matmul(
                            psums[pi][:], w_sb[:, h, 128 * i:128 * (i + 1)],
                            xt_b[:, h, :],
                            start=(h == 0), stop=(h == NH_T - 1))
                return psums

            def swap1(c, ps, on_act=False):
                # psum->SBUF copy + partition half-swap for one rope tensor
                if on_act:
                    nc.scalar.copy(qkv_sb[:, c, tsl], ps[:])
                else:
                    nc.vector.tensor_copy(qkv_sb[:, c, tsl], ps[:])
                xs = rp.tile([128, 512], BF16, tag=f"xs1_{c}",
                             name=f"xs1_{g}_{c}")
                nc.sync.dma_start(out=xs[0:64, :], in_=qkv_sb[64:128, c, tsl])
                nc.sync.dma_start(out=xs[64:128, :], in_=qkv_sb[0:64, c, tsl])
                return xs

            def rope1(c, xs):
                x = qkv_sb[:, c, tsl]
                t1 = rp.tile([128, 512], BF16, tag="t1", name=f"t1_{g}_{c}")
                t2 = rp.tile([128, 512], BF16, tag="t2", name=f"t2_{g}_{c}")
                nc.vector.tensor_mul(t1[:], x, cdup[:, tsl])
                nc.vector.tensor_mul(t2[:], xs[:], sdup[:, tsl])
                nc.vector.tensor_add(x, t1[:], t2[:])

            if g == NG - 1:
                # last chunk: project (k, q0) first, rope them, and pre-emit
                # the first NPRE score+exp tiles of head 0 so ACT starts the
                # tail's exp stream (the end-time critical chain) while the
                # (v, q1) pass still runs on the PE. No AVs here: they would
                # deadlock the PE FIFO on the V transpose behind pass 1.
                ps0 = qkv_pass((2, 0), 0)
                xs_k1 = swap1(2, ps0[0])
                xs_q0 = swap1(0, ps0[1], on_act=True)
                rope1(2, xs_k1)
                rope1(0, xs_q0)
                for t in range(1, NPRE + 1):
                    emit_score(g, 0, t, pre_es)
                ps1 = qkv_pass((3, 1), 1)
                nc.vector.tensor_copy(v_sb[:, tsl], ps1[0][:])
                nc.sync.dma_start_transpose(
                    out=V_sb[:, 4 * g:4 * (g + 1), :], in_=v_sb[:, tsl])
                xs_q1 = swap1(1, ps1[1], on_act=True)
                rope1(1, xs_q1)
            elif g == 0:
                # q0/q1 psums borrow attention slots; k/v take the qkvp
                # banks, whose copies run first -- so chunk 1's projection
                # (which reuses qkvp) unblocks as early as possible
                psums = [spp.tile([128, 512], F32, tag="sp",
                                  name=f"qkvps_0_b_{i}") for i in range(2)]
                psums += [qkvp.tile([128, 512], F32, tag="qkvps",
                                    name=f"qkvps_0_a_{i}") for i in range(2)]
                for h in range(NH_T):
                    for i in range(4):
                        nc.tensor.matmul(
                            psums[i][:], w_sb[:, h, 128 * i:128 * (i + 1)],
                            xt_b[:, h, :],
                            start=(h == 0), stop=(h == NH_T - 1))
                xs_k = copy_kv(psums[2], psums[3])
                xs_q = copy_q(psums[0], psums[1])
            else:
                for p in range(2):
                    psums = [qkvp.tile([128, 512], F32, tag="qkvps",
                                       name=f"qkvps_{g}_{p}_{i}")
                             for i in range(2)]
                    cols = (2, 3) if p == 0 else (0, 1)
                    for h in range(NH_T):
                        for pi, i in enumerate(cols):
                            nc.tensor.matmul(
                                psums[pi][:], w_sb[:, h, 128 * i:128 * (i + 1)],
                                xt_b[:, h, :],
                                start=(h == 0), stop=(h == NH_T - 1))
                    if p == 0:
                        xs_k = copy_kv(psums[0], psums[1])
                    else:
                        xs_q = copy_q(psums[0], psums[1])

            if g < NG - 1:
                # ---- V transpose via the DMA XBAR (bf16): no PE work, no
                # psum contention with o_proj, no DVE copies
                nc.sync.dma_start_transpose(
                    out=V_sb[:, 4 * g:4 * (g + 1), :], in_=v_sb[:, tsl])

                # ---- rope (k first: it gates the next chunk's scores)
                for t3 in (2, 0, 1):
                    x = qkv_sb[:, t3, tsl]
                    xs = xs_k[:, :] if t3 == 2 else xs_q[:, t3, :]
                    t1 = rp.tile([128, 512], BF16, tag="t1",
                                 name=f"t1_{g}_{t3}")
                    t2 = rp.tile([128, 512], BF16, tag="t2",
                                 name=f"t2_{g}_{t3}")
                    nc.vector.tensor_mul(t1[:], x, cdup[:, tsl])
                    nc.vector.tensor_mul(t2[:], xs, sdup[:, tsl])
                    nc.vector.tensor_add(x, t1[:], t2[:])

            if prev_ras is not None and g < NG - 1:
                for th in oproj_ops(g - 1, copies_on_act=False):
                    th()
            prev_ras = g

        # tail: chunk NG-2's o_proj is interleaved into chunk NG-1's
        # attention (which is otherwise exp-throughput-bound with nothing
        # else to run); its psums use the now-free qkv banks
        fill = iter(oproj_ops(NG - 2, psum_pool=qkvp, psum_tag="qkvps",
                              copies_on_act=False))
        def on_head(h, entry):
            # head 0's normalize chain emitted between the heads: its ones/
            # recip/mul drain under head 1's attention, so o_proj's LEAD
            # stream starts the moment the last AV lands
            if h == 0:
                finish_head(NG - 1, 0, entry)
        ras = attn(NG - 1, filler=fill, pre={0: pre_es}, on_head=on_head)
        for th in fill:
            th()
        finish_head(NG - 1, 1, ras[1])
        oproj(NG - 1)

    nc.compile()
    return nc


_NC_CACHE = None


def _get_nc():
    global _NC_CACHE
    if _NC_CACHE is None:
        _NC_CACHE = _build()
    return _NC_CACHE


def _host_prep(positions, hidden_states, w_qkv, w_o):
    positions = np.asarray(positions, dtype=np.int32)
    hidden_states = np.asarray(hidden_states, dtype=np.float32)
    w_qkv = np.asarray(w_qkv, dtype=np.float32)
    w_o = np.asarray(w_o, dtype=np.float32)
    bf = ml_dtypes.bfloat16

    xT = np.ascontiguousarray(hidden_states.T).astype(bf)

    # rope tables: partition p holds pair p%64; lower half is the x1 (even)
    # feature, upper half the x2 (odd) feature of each rotary pair
    ang = positions[ROW_MAP, :].astype(np.float64) * INVF[:, None]
    cos = np.cos(ang).astype(np.float32)
    sin = np.sin(ang).astype(np.float32)
    cdup = np.ascontiguousarray(np.concatenate([cos, cos], axis=0)).astype(bf)
    sdup = np.ascontiguousarray(np.concatenate([-sin, sin], axis=0)).astype(bf)

    # additive causal mask factors: invalid(dk, dq) = [dq - 128m + 1 <= dk]
    #   = sum_p L[p, dk] * Rm[p, dq],  L[p, dk] = [p <= dk],
    #   Rm[p, dq] = [p == max(dq - 128m + 1, 0)]  (scaled by -1e9)
    mask_l = (np.arange(128)[:, None] <= np.arange(128)[None, :]).astype(np.float32)
    mask_r = np.zeros((128, 4, 512), dtype=np.float32)
    for m in range(4):
        c = np.maximum(np.arange(512) - 128 * m + 1, 0)
        valid_rows = c <= 127
        mask_r[c[valid_rows], m, np.arange(512)[valid_rows]] = -1e9
    ones = np.ones((128, 128), dtype=np.float32)

    q_size = N_HEADS * HD
    kv_size = N_KV * HD
    in_maps = []
    for c in range(NCORES):
        cols = [w_qkv[:, 2 * c * HD + PERM], w_qkv[:, (2 * c + 1) * HD + PERM]]
        kc = c // 2
        cols.append(w_qkv[:, q_size + kc * HD + PERM])
        cols.append(w_qkv[:, q_size + kv_size + kc * HD:q_size + kv_size + (kc + 1) * HD])
        w_slice = np.ascontiguousarray(np.concatenate(cols, axis=1)).astype(bf)
        wo_slice = np.ascontiguousarray(w_o[2 * c * HD:(2 * c + 2) * HD]).astype(bf)
        in_maps.append({
            "xT": xT, "w_slice": w_slice, "wo_slice": wo_slice,
            "cdup": cdup, "sdup": sdup,
            "mask_l": mask_l.astype(bf), "mask_r": mask_r.astype(bf),
            "ones": ones.astype(bf),
        })
    return in_maps


def kernel(positions, hidden_states, w_qkv, w_o):
    nc = _get_nc()
    in_maps = _host_prep(positions, hidden_states, w_qkv, w_o)
    # one retry: transient NRT/device errors (e.g. NRT_EXEC_UNIT_UNRECOVERABLE
    # from a wedged core) were observed to succeed on re-dispatch
    try:
        res = run_bass_kernel_spmd(nc, in_maps, core_ids=list(range(NCORES)))
    except Exception:
        import time
        time.sleep(2.0)
        res = run_bass_kernel_spmd(nc, in_maps, core_ids=list(range(NCORES)))
    yT = np.zeros((HIDDEN, T), dtype=np.float64)
    for c in range(NCORES):
        yT += res.results[c]["yT"].astype(np.float64)
    return np.ascontiguousarray(yT.T).astype(np.float32)
